# revision 17
# baseline (speedup 1.0000x reference)
"""Trainium2 Bass kernel: FiLM-conditioned 1x1-conv self-attention block.

Sharding: 8 cores = 2 batches x 4 heads. Each core computes one (batch, head)
pair end-to-end, producing a partial output projection [C, N]; the host sums
the 4 head partials per batch (b_out is added on the h==0 cores only).

Math notes (all exact re-associations of the reference):
  - FiLM: x~ = x*(1+scale) + shift  =>  W @ x~ = (W*diag(1+scale)) @ x + (W@shift)
    so the modulation is folded into the QKV weight columns + a rank-1 bias.
  - Attention is computed in transposed layout S^T[j,i] = sum_d k[d,j] q[d,i];
    softmax scale 1/sqrt(d) is folded into the exp activation's free scale.
    No max-subtraction (scores are O(5), exp is safe in fp32).
  - U = [V; 1]^T @ P^T accumulated over j gives both the unnormalized output
    (rows 0..32) and the softmax denominator (row 32) in one matmul chain.
"""

import functools
import sys

import numpy as np

if "/opt/trn_rl_repo" not in sys.path:
    sys.path.insert(0, "/opt/trn_rl_repo")

HEADS = 4
D = 32              # dim head
C = 256             # channels
TD = 512            # time embedding dim
SCALE = D ** -0.5
N_FULL = 4096       # 64*64 spatial positions
NT = 512            # query (i) tile
JT = 128            # key (j) tile
N_CORES = 8


def _build_program(n_pos=N_FULL):
    import concourse.bass as bass
    import concourse.tile as tile
    from concourse import bacc, mybir
    from concourse.masks import make_identity

    f32 = mybir.dt.float32
    bf16 = mybir.dt.bfloat16
    AF = mybir.ActivationFunctionType

    nc = bacc.Bacc("TRN2", debug=False)

    xb = nc.dram_tensor("xb", [C, n_pos], f32, kind="ExternalInput").ap()
    te = nc.dram_tensor("te", [TD], f32, kind="ExternalInput").ap()
    w_mlp = nc.dram_tensor("w_mlp", [TD, TD], f32, kind="ExternalInput").ap()
    b_mlp = nc.dram_tensor("b_mlp", [TD], f32, kind="ExternalInput").ap()
    wq = nc.dram_tensor("wq", [D, C], f32, kind="ExternalInput").ap()
    wk = nc.dram_tensor("wk", [D, C], f32, kind="ExternalInput").ap()
    wv = nc.dram_tensor("wv", [D, C], f32, kind="ExternalInput").ap()
    wo = nc.dram_tensor("wo", [C, D], f32, kind="ExternalInput").ap()
    bo = nc.dram_tensor("bo", [C], f32, kind="ExternalInput").ap()
    out = nc.dram_tensor("out", [C, n_pos], f32, kind="ExternalOutput").ap()

    n_itiles = n_pos // NT
    n_jtiles = n_pos // JT
    n_jpairs = n_jtiles // 2

    with tile.TileContext(nc) as tc:
        with tc.tile_pool(name="const", bufs=1) as const:
            ident = const.tile([128, 128], f32, name="ident")
            make_identity(nc, ident)
            ident_bf = const.tile([128, 128], bf16, name="ident_bf")
            make_identity(nc, ident_bf)

            # persistent big tiles
            x_sb = []
            for cc in range(2):
                xt = const.tile([128, n_pos], bf16, name=f"x_sb{cc}")
                x_sb.append(xt)
            q4 = const.tile([128, n_pos], bf16, name="q4")
            k4 = const.tile([128, n_pos], bf16, name="k4")
            v_sb = const.tile([D, n_pos], bf16, name="v_sb")

            # FiLM results
            tfull = const.tile([128, 4], f32, name="tfull")
            sc1 = const.tile([128, 2], f32, name="sc1")
            # folded qkv weights (transposed, q/k replicated 4x along M)
            q4T = [const.tile([128, 128], bf16, name=f"q4T{cc}") for cc in range(2)]
            k4T = [const.tile([128, 128], bf16, name=f"k4T{cc}") for cc in range(2)]
            vT_w = [const.tile([128, D], bf16, name=f"vT_w{cc}") for cc in range(2)]
            bq4 = const.tile([128, 1], f32, name="bq4")
            bk4 = const.tile([128, 1], f32, name="bk4")
            bv = const.tile([D, 1], f32, name="bv")
            woT = [const.tile([D, 128], bf16, name=f"woT{hh}") for hh in range(2)]
            bo_t = const.tile([128, 2], f32, name="bo_t")
            vt_tiles = [
                const.tile([128, D + 1], bf16, name=f"vt_{j}") for j in range(n_jtiles)
            ]

            # ---------------- prologue: FiLM + weight prep ----------------
            with tc.tile_pool(name="pro_sb", bufs=3) as pro_sb, \
                 tc.tile_pool(name="pro_ps", bufs=2, space="PSUM") as pro_ps:

                # x loads (fp32 staging -> bf16 convert)
                for cc in range(2):
                    for qq in range(0, n_pos, 1024):
                        w = min(1024, n_pos - qq)
                        x_stage = pro_sb.tile([128, 1024], f32, tag="x_stage",
                                              bufs=3, name=f"x_stage_{cc}_{qq}")
                        nc.sync.dma_start(
                            out=x_stage[:, :w],
                            in_=xb[128 * cc:128 * (cc + 1), qq:qq + w],
                        )
                        nc.vector.tensor_copy(x_sb[cc][:, qq:qq + w], x_stage[:, :w])

                # silu(time_emb) chunked [128, 4] (c = 128*f + p)
                te_t = pro_sb.tile([128, 4], f32, tag="te_t")
                nc.sync.dma_start(out=te_t, in_=te.rearrange("(f p) -> p f", p=128))
                s_t = const.tile([128, 4], f32, name="s_t")
                sig_t = pro_sb.tile([128, 4], f32, tag="sig_t")
                nc.scalar.activation(sig_t, te_t, AF.Sigmoid)
                nc.vector.tensor_mul(s_t, te_t, sig_t)

                # W_mlp^T tiles via PE transpose
                wmT = [[None] * 4 for _ in range(4)]
                for cc4 in range(4):
                    for ot in range(4):
                        wm_nat = pro_sb.tile([128, 128], f32, tag="wm_nat")
                        nc.sync.dma_start(
                            out=wm_nat,
                            in_=w_mlp[ot * 128:(ot + 1) * 128,
                                      cc4 * 128:(cc4 + 1) * 128],
                        )
                        ps_t = pro_ps.tile([128, 128], f32, tag="tp", bufs=3)
                        nc.tensor.transpose(ps_t, wm_nat, ident)
                        wmT_t = pro_sb.tile([128, 128], f32, tag=f"wmT_{cc4}_{ot}",
                                            name=f"wmT_{cc4}_{ot}", bufs=1)
                        nc.vector.tensor_copy(wmT_t, ps_t)
                        wmT[cc4][ot] = wmT_t

                # t = W_mlp @ silu(te)  (o-tile at a time), + b_mlp
                bm_t = pro_sb.tile([128, 4], f32, tag="bm_t")
                nc.sync.dma_start(out=bm_t, in_=b_mlp.rearrange("(f p) -> p f", p=128))
                for ot in range(4):
                    t_ps = pro_ps.tile([128, 1], f32, tag="t_ps", bufs=2)
                    for cc4 in range(4):
                        nc.tensor.matmul(
                            t_ps, wmT[cc4][ot], s_t[:, cc4:cc4 + 1],
                            start=(cc4 == 0), stop=(cc4 == 3),
                        )
                    nc.vector.tensor_add(
                        tfull[:, ot:ot + 1], t_ps, bm_t[:, ot:ot + 1]
                    )
                # scale+1 for c-chunks (cols 0,1); shift is cols 2,3
                nc.vector.tensor_scalar_add(sc1, tfull[:, 0:2], 1.0)

                # W_qkv head slices -> transpose -> scale cols by (1+scale)
                for name, wsrc, dstT, nrep in (
                    ("q", wq, q4T, 4), ("k", wk, k4T, 4), ("v", wv, vT_w, 1),
                ):
                    w_nat = pro_sb.tile([D, C], f32, tag=f"w_nat_{name}", bufs=1)
                    nc.sync.dma_start(out=w_nat, in_=wsrc)
                    uT = [None, None]
                    for cc in range(2):
                        ps_t = pro_ps.tile([128, D], f32, tag="tp", bufs=3)
                        nc.tensor.transpose(
                            ps_t, w_nat[:, 128 * cc:128 * (cc + 1)],
                            ident[0:D, 0:D],
                        )
                        # unscaled copy (for the shift-bias matmul)
                        uT_t = pro_sb.tile([128, D], f32, tag=f"uT_{name}{cc}",
                                           name=f"uT_{name}{cc}", bufs=1)
                        nc.vector.tensor_copy(uT_t, ps_t)
                        uT[cc] = uT_t
                        # scaled (FiLM) copies, replicated nrep x along M
                        for r in range(nrep):
                            nc.vector.tensor_scalar_mul(
                                dstT[cc][:, D * r:D * (r + 1)], ps_t,
                                sc1[:, cc:cc + 1],
                            )
                    # bias_g = W_g @ shift (unscaled weights)
                    b_ps = pro_ps.tile([D, 1], f32, tag="b_ps", bufs=2)
                    for cc in range(2):
                        nc.tensor.matmul(
                            b_ps, uT[cc], tfull[:, 2 + cc:3 + cc],
                            start=(cc == 0), stop=(cc == 1),
                        )
                    if name == "q":
                        for r in range(4):
                            nc.vector.tensor_copy(bq4[D * r:D * (r + 1), :], b_ps)
                    elif name == "k":
                        for r in range(4):
                            nc.vector.tensor_copy(bk4[D * r:D * (r + 1), :], b_ps)
                    else:
                        nc.vector.tensor_copy(bv, b_ps)

                # w_out^T halves
                for hh in range(2):
                    wo_nat = pro_sb.tile([128, D], f32, tag="wo_nat")
                    nc.sync.dma_start(
                        out=wo_nat, in_=wo[128 * hh:128 * (hh + 1), :]
                    )
                    ps_t = pro_ps.tile([D, 128], f32, tag="tp", bufs=3)
                    nc.tensor.transpose(ps_t, wo_nat, ident)
                    nc.vector.tensor_copy(woT[hh], ps_t)  # fp32 psum -> bf16
                bo_dma = pro_sb.tile([128, 2], f32, tag="bo_dma")
                nc.sync.dma_start(out=bo_dma, in_=bo.rearrange("(f p) -> p f", p=128))
                nc.vector.tensor_copy(bo_t, bo_dma)

            # ---------------- QKV projection + V^T build ----------------
            with tc.tile_pool(name="qkv_ps", bufs=2, space="PSUM") as qkv_ps, \
                 tc.tile_pool(name="vt_ps", bufs=2, space="PSUM") as vt_ps:
                for nt in range(0, n_pos, NT):
                    sl = slice(nt, nt + NT)
                    ps_q = qkv_ps.tile([128, NT], f32, tag="q")
                    ps_k = qkv_ps.tile([128, NT], f32, tag="k")
                    ps_v = qkv_ps.tile([D, NT], f32, tag="v")
                    for cc in range(2):
                        st, sp = (cc == 0), (cc == 1)
                        nc.tensor.matmul(ps_q, q4T[cc], x_sb[cc][:, sl], start=st, stop=sp)
                        nc.tensor.matmul(ps_k, k4T[cc], x_sb[cc][:, sl], start=st, stop=sp)
                        nc.tensor.matmul(ps_v, vT_w[cc], x_sb[cc][:, sl], start=st, stop=sp)
                    nc.vector.tensor_scalar_add(q4[:, sl], ps_q, bq4)
                    nc.vector.tensor_scalar_add(k4[:, sl], ps_k, bk4)
                    nc.vector.tensor_scalar_add(v_sb[:, sl], ps_v, bv)
                    # V^T tiles for the 4 j-tiles covered by this n-tile
                    for j in range(nt // JT, (nt + NT) // JT):
                        ps_vt = vt_ps.tile([128, D], bf16, tag="vt")
                        nc.tensor.transpose(
                            ps_vt, v_sb[:, j * JT:(j + 1) * JT], ident_bf[0:D, 0:D]
                        )
                        nc.vector.tensor_copy(vt_tiles[j][:, 0:D], ps_vt)
                        nc.vector.memset(vt_tiles[j][:, D:D + 1], 1.0)

            # ---------------- attention + output projection ----------------
            with tc.tile_pool(name="sc_ps", bufs=2, space="PSUM") as sc_ps, \
                 tc.tile_pool(name="u_ps", bufs=1, space="PSUM") as u_ps, \
                 tc.tile_pool(name="aux_ps", bufs=2, space="PSUM") as aux_ps, \
                 tc.tile_pool(name="pt_sb", bufs=4) as pt_sb, \
                 tc.tile_pool(name="o_sb", bufs=2) as o_sb:
                def emit_proj(pit, onorm_t):
                    psl = slice(pit * NT, (pit + 1) * NT)
                    for hh in range(2):
                        ps_o = aux_ps.tile([128, NT], f32, tag="aux",
                                           name=f"ps_o_{pit}_{hh}")
                        nc.tensor.matmul(ps_o, woT[hh], onorm_t,
                                         start=True, stop=True)
                        o_out = o_sb.tile([128, NT], f32, tag="o_out",
                                          name=f"o_out_{pit}_{hh}")
                        nc.vector.tensor_scalar_add(o_out, ps_o, bo_t[:, hh:hh + 1])
                        nc.sync.dma_start(
                            out=out[128 * hh:128 * (hh + 1), psl], in_=o_out
                        )

                pending = None
                for it in range(n_itiles):
                    isl = slice(it * NT, (it + 1) * NT)
                    U_a = u_ps.tile([D + 1, NT], f32, tag="ua")
                    U_b = u_ps.tile([97, NT], f32, tag="ub")

                    def emit_scores(jp):
                        jA, jB = 2 * jp, 2 * jp + 1
                        S = sc_ps.tile([128, 2 * NT], f32, tag="sc", name=f"S_{jp}")
                        nc.tensor.matmul(
                            S[:, 0:NT],
                            k4[0:D, jA * JT:(jA + 1) * JT],
                            q4[0:D, isl],
                            start=True, stop=True, tile_position=(0, 0),
                        )
                        nc.tensor.matmul(
                            S[:, NT:2 * NT],
                            k4[D:2 * D, jB * JT:(jB + 1) * JT],
                            q4[D:2 * D, isl],
                            start=True, stop=True, tile_position=(32, 0),
                        )
                        PT = pt_sb.tile([128, 2 * NT], bf16, tag="pt",
                                        name=f"PT_{jp}")
                        nc.scalar.activation(PT, S, AF.Exp, scale=SCALE)
                        return PT

                    def emit_u(jp, PT):
                        jA, jB = 2 * jp, 2 * jp + 1
                        st, sp = (jp == 0), (jp == n_jpairs - 1)
                        nc.tensor.matmul(
                            U_a, vt_tiles[jA], PT[:, 0:NT],
                            start=st, stop=sp, tile_position=(0, 0),
                        )
                        nc.tensor.matmul(
                            U_b[64:64 + D + 1, :], vt_tiles[jB], PT[:, NT:2 * NT],
                            start=st, stop=sp, tile_position=(0, 64),
                        )

                    # software pipeline: scores run one stage ahead of U so
                    # the strict-FIFO PE queue never blocks a ready S matmul
                    # behind a U matmul whose exp isn't done yet.
                    prev_pt = emit_scores(0)
                    for jp in range(1, n_jpairs):
                        pt_cur = emit_scores(jp)
                        emit_u(jp - 1, prev_pt)
                        prev_pt = pt_cur
                    emit_u(n_jpairs - 1, prev_pt)
                    # combine halves + normalize (DVE/GpSimd, overlaps the
                    # next i-tile's j-loop); projection is deferred one i-tile
                    # so its PE matmuls never stall on this chain.
                    usum_b = o_sb.tile([D + 1, NT], f32, tag="usum_b")
                    nc.vector.tensor_copy(usum_b, U_b[64:64 + D + 1, :])
                    usum = o_sb.tile([D + 1, NT], f32, tag="usum")
                    nc.vector.tensor_add(usum, U_a, usum_b)
                    rcp = o_sb.tile([1, NT], f32, tag="rcp")
                    nc.vector.reciprocal(rcp, usum[D:D + 1, :])
                    rrep = o_sb.tile([D, NT], f32, tag="rrep")
                    nc.gpsimd.partition_broadcast(rrep, rcp)
                    onorm = o_sb.tile([D, NT], bf16, tag="onorm")
                    nc.vector.tensor_mul(onorm, usum[0:D, :], rrep)
                    if pending is not None:
                        emit_proj(*pending)
                    pending = (it, onorm)
                if pending is not None:
                    emit_proj(*pending)
    nc.compile()
    return nc


@functools.lru_cache(maxsize=2)
def _get_nc(n_pos=N_FULL):
    return _build_program(n_pos)


def _make_in_maps(x, time_emb, w_mlp, b_mlp, w_qkv, w_out, b_out, n_pos=N_FULL):
    x = np.ascontiguousarray(np.asarray(x, dtype=np.float32))
    time_emb = np.ascontiguousarray(np.asarray(time_emb, dtype=np.float32))
    w_mlp = np.ascontiguousarray(np.asarray(w_mlp, dtype=np.float32))
    b_mlp = np.ascontiguousarray(np.asarray(b_mlp, dtype=np.float32))
    w_qkv = np.ascontiguousarray(np.asarray(w_qkv, dtype=np.float32))
    w_out = np.ascontiguousarray(np.asarray(w_out, dtype=np.float32))
    b_out = np.ascontiguousarray(np.asarray(b_out, dtype=np.float32))

    b = x.shape[0]
    hid = HEADS * D
    in_maps = []
    for core in range(N_CORES):
        bb, hh = core // HEADS, core % HEADS
        in_maps.append({
            "xb": np.ascontiguousarray(
                x[bb].reshape(C, -1)[:, :n_pos]),
            "te": time_emb[bb],
            "w_mlp": w_mlp,
            "b_mlp": b_mlp,
            "wq": np.ascontiguousarray(w_qkv[D * hh:D * (hh + 1), :]),
            "wk": np.ascontiguousarray(w_qkv[hid + D * hh:hid + D * (hh + 1), :]),
            "wv": np.ascontiguousarray(
                w_qkv[2 * hid + D * hh:2 * hid + D * (hh + 1), :]),
            "wo": np.ascontiguousarray(w_out[:, D * hh:D * (hh + 1)]),
            "bo": b_out if hh == 0 else np.zeros_like(b_out),
        })
    return in_maps


def _install_ntff_hook():
    """Register the axon NTFF profile hook (the agent image's antenv lacks
    axon_hooks; replicate trn_boot's ctypes shim so trace=True works)."""
    import types
    import contextlib
    import ctypes

    try:
        from antenv.axon_hooks import get_axon_ntff_profile_hook  # noqa: F401
        return
    except ImportError:
        pass
    so_path = "/opt/axon/libaxon_pjrt.so"
    try:
        lib = ctypes.CDLL(so_path)
    except OSError:
        return
    if not hasattr(lib, "axon_start_nrt_profile"):
        return
    lib.axon_start_nrt_profile.argtypes = [
        ctypes.POINTER(ctypes.c_int64), ctypes.c_size_t]
    lib.axon_start_nrt_profile.restype = ctypes.c_int64
    lib.axon_stop_nrt_profile.argtypes = [ctypes.c_char_p]
    lib.axon_stop_nrt_profile.restype = ctypes.c_int64

    @contextlib.contextmanager
    def _hook(output_dir, device_ids):
        import jax
        jax.devices()
        if device_ids:
            ids = (ctypes.c_int64 * len(device_ids))(*device_ids)
            rc = lib.axon_start_nrt_profile(ids, len(device_ids))
        else:
            rc = lib.axon_start_nrt_profile(None, 0)
        if rc != 0:
            raise RuntimeError(f"axon_start_nrt_profile rc={rc}")
        try:
            yield
        finally:
            n = lib.axon_stop_nrt_profile(str(output_dir).encode())
            print(f"profile: {n} file(s) written to {output_dir}",
                  file=sys.stderr)

    import antenv
    mod = types.ModuleType("antenv.axon_hooks")
    mod.get_axon_ntff_profile_hook = lambda: _hook
    mod.set_axon_ntff_profile_hook = lambda h: None
    sys.modules["antenv.axon_hooks"] = mod
    antenv.axon_hooks = mod


def _run(inputs, trace=False, n_pos=N_FULL):
    from concourse.bass_utils import run_bass_kernel_spmd

    if trace:
        _install_ntff_hook()
    nc = _get_nc(n_pos)
    in_maps = _make_in_maps(**inputs, n_pos=n_pos)
    res = run_bass_kernel_spmd(
        nc, in_maps, core_ids=list(range(N_CORES)), trace=trace
    )
    return res


def _assemble(results, x_shape):
    b, c, h, w = x_shape
    out = np.zeros((b, c, h * w), dtype=np.float32)
    for core in range(N_CORES):
        bb = core // HEADS
        out[bb] += results[core]["out"]
    return out.reshape(b, c, h, w)


def kernel(x, time_emb, w_mlp, b_mlp, w_qkv, w_out, b_out):
    res = _run(dict(
        x=x, time_emb=time_emb, w_mlp=w_mlp, b_mlp=b_mlp,
        w_qkv=w_qkv, w_out=w_out, b_out=b_out,
    ))
    return _assemble(res.results, np.asarray(x).shape)


# revision 18
# speedup vs baseline: 1.0626x; 1.0626x over previous
"""Trainium2 Bass kernel: FiLM-conditioned 1x1-conv self-attention block.

Sharding: 8 cores = 2 batches x 4 heads. Each core computes one (batch, head)
pair end-to-end, producing a partial output projection [C, N]; the host sums
the 4 head partials per batch (b_out is added on the h==0 cores only).

Math notes (all exact re-associations of the reference):
  - FiLM: x~ = x*(1+scale) + shift  =>  W @ x~ = (W*diag(1+scale)) @ x + (W@shift)
    so the modulation is folded into the QKV weight columns + a rank-1 bias.
  - Attention is computed in transposed layout S^T[j,i] = sum_d k[d,j] q[d,i];
    softmax scale 1/sqrt(d) is folded into the exp activation's free scale.
    No max-subtraction (scores are O(5), exp is safe in fp32).
  - U = [V; 1]^T @ P^T accumulated over j gives both the unnormalized output
    (rows 0..32) and the softmax denominator (row 32) in one matmul chain.
"""

import functools
import sys

import numpy as np

if "/opt/trn_rl_repo" not in sys.path:
    sys.path.insert(0, "/opt/trn_rl_repo")

HEADS = 4
D = 32              # dim head
C = 256             # channels
TD = 512            # time embedding dim
SCALE = D ** -0.5
N_FULL = 4096       # 64*64 spatial positions
NT = 512            # query (i) tile
JT = 128            # key (j) tile
N_CORES = 8


def _build_program(n_pos=N_FULL):
    import concourse.bass as bass
    import concourse.tile as tile
    from concourse import bacc, mybir
    from concourse.masks import make_identity

    f32 = mybir.dt.float32
    bf16 = mybir.dt.bfloat16
    AF = mybir.ActivationFunctionType

    nc = bacc.Bacc("TRN2", debug=False)

    xb = nc.dram_tensor("xb", [C, n_pos], f32, kind="ExternalInput").ap()
    te = nc.dram_tensor("te", [TD], f32, kind="ExternalInput").ap()
    w_mlp = nc.dram_tensor("w_mlp", [TD, TD], f32, kind="ExternalInput").ap()
    b_mlp = nc.dram_tensor("b_mlp", [TD], f32, kind="ExternalInput").ap()
    wq = nc.dram_tensor("wq", [D, C], f32, kind="ExternalInput").ap()
    wk = nc.dram_tensor("wk", [D, C], f32, kind="ExternalInput").ap()
    wv = nc.dram_tensor("wv", [D, C], f32, kind="ExternalInput").ap()
    wo = nc.dram_tensor("wo", [C, D], f32, kind="ExternalInput").ap()
    bo = nc.dram_tensor("bo", [C], f32, kind="ExternalInput").ap()
    out = nc.dram_tensor("out", [C, n_pos], f32, kind="ExternalOutput").ap()

    n_itiles = n_pos // NT
    n_jtiles = n_pos // JT
    n_jpairs = n_jtiles // 2

    with tile.TileContext(nc) as tc:
        with tc.tile_pool(name="const", bufs=1) as const:
            ident = const.tile([128, 128], f32, name="ident")
            make_identity(nc, ident)
            ident_bf = const.tile([128, 128], bf16, name="ident_bf")
            make_identity(nc, ident_bf)

            # persistent big tiles
            x_sb = []
            for cc in range(2):
                xt = const.tile([128, n_pos], bf16, name=f"x_sb{cc}")
                x_sb.append(xt)
            q4 = const.tile([128, n_pos], bf16, name="q4")
            k4 = const.tile([128, n_pos], bf16, name="k4")
            v_sb = const.tile([D, n_pos], bf16, name="v_sb")

            # FiLM results
            tfull = const.tile([128, 4], f32, name="tfull")
            sc1 = const.tile([128, 2], f32, name="sc1")
            # folded qkv weights (transposed, q/k replicated 4x along M)
            q4T = [const.tile([128, 128], bf16, name=f"q4T{cc}") for cc in range(2)]
            k4T = [const.tile([128, 128], bf16, name=f"k4T{cc}") for cc in range(2)]
            vT_w = [const.tile([128, D], bf16, name=f"vT_w{cc}") for cc in range(2)]
            bq4 = const.tile([128, 1], f32, name="bq4")
            bk4 = const.tile([128, 1], f32, name="bk4")
            bv = const.tile([D, 1], f32, name="bv")
            woT = [const.tile([D, 128], bf16, name=f"woT{hh}") for hh in range(2)]
            bo_t = const.tile([128, 2], f32, name="bo_t")
            vt_tiles = [
                const.tile([128, D + 1], bf16, name=f"vt_{j}") for j in range(n_jtiles)
            ]

            # ---------------- prologue: FiLM + weight prep ----------------
            with tc.tile_pool(name="pro_sb", bufs=3) as pro_sb, \
                 tc.tile_pool(name="pro_ps", bufs=2, space="PSUM") as pro_ps:

                # silu(time_emb) chunked [128, 4] (c = 128*f + p)
                te_t = pro_sb.tile([128, 4], f32, tag="te_t")
                nc.sync.dma_start(out=te_t, in_=te.rearrange("(f p) -> p f", p=128))
                s_t = const.tile([128, 4], f32, name="s_t")
                sig_t = pro_sb.tile([128, 4], f32, tag="sig_t")
                nc.scalar.activation(sig_t, te_t, AF.Sigmoid)
                nc.vector.tensor_mul(s_t, te_t, sig_t)

                # W_mlp^T tiles via PE transpose
                wmT = [[None] * 4 for _ in range(4)]
                for cc4 in range(4):
                    for ot in range(4):
                        wm_nat = pro_sb.tile([128, 128], f32, tag="wm_nat")
                        nc.sync.dma_start(
                            out=wm_nat,
                            in_=w_mlp[ot * 128:(ot + 1) * 128,
                                      cc4 * 128:(cc4 + 1) * 128],
                        )
                        ps_t = pro_ps.tile([128, 128], f32, tag="tp", bufs=3)
                        nc.tensor.transpose(ps_t, wm_nat, ident)
                        wmT_t = pro_sb.tile([128, 128], f32, tag=f"wmT_{cc4}_{ot}",
                                            name=f"wmT_{cc4}_{ot}", bufs=1)
                        nc.vector.tensor_copy(wmT_t, ps_t)
                        wmT[cc4][ot] = wmT_t

                # t = W_mlp @ silu(te)  (o-tile at a time), + b_mlp
                bm_t = pro_sb.tile([128, 4], f32, tag="bm_t")
                nc.sync.dma_start(out=bm_t, in_=b_mlp.rearrange("(f p) -> p f", p=128))
                for ot in range(4):
                    t_ps = pro_ps.tile([128, 1], f32, tag="t_ps", bufs=2)
                    for cc4 in range(4):
                        nc.tensor.matmul(
                            t_ps, wmT[cc4][ot], s_t[:, cc4:cc4 + 1],
                            start=(cc4 == 0), stop=(cc4 == 3),
                        )
                    nc.vector.tensor_add(
                        tfull[:, ot:ot + 1], t_ps, bm_t[:, ot:ot + 1]
                    )
                # scale+1 for c-chunks (cols 0,1); shift is cols 2,3
                nc.vector.tensor_scalar_add(sc1, tfull[:, 0:2], 1.0)

                # W_qkv head slices -> transpose -> scale cols by (1+scale)
                for name, wsrc, dstT, nrep in (
                    ("q", wq, q4T, 4), ("k", wk, k4T, 4), ("v", wv, vT_w, 1),
                ):
                    w_nat = pro_sb.tile([D, C], f32, tag=f"w_nat_{name}", bufs=1)
                    nc.sync.dma_start(out=w_nat, in_=wsrc)
                    uT = [None, None]
                    for cc in range(2):
                        ps_t = pro_ps.tile([128, D], f32, tag="tp", bufs=3)
                        nc.tensor.transpose(
                            ps_t, w_nat[:, 128 * cc:128 * (cc + 1)],
                            ident[0:D, 0:D],
                        )
                        # unscaled copy (for the shift-bias matmul)
                        uT_t = pro_sb.tile([128, D], f32, tag=f"uT_{name}{cc}",
                                           name=f"uT_{name}{cc}", bufs=1)
                        nc.vector.tensor_copy(uT_t, ps_t)
                        uT[cc] = uT_t
                        # scaled (FiLM) copies, replicated nrep x along M
                        for r in range(nrep):
                            nc.vector.tensor_scalar_mul(
                                dstT[cc][:, D * r:D * (r + 1)], ps_t,
                                sc1[:, cc:cc + 1],
                            )
                    # bias_g = W_g @ shift (unscaled weights)
                    b_ps = pro_ps.tile([D, 1], f32, tag="b_ps", bufs=2)
                    for cc in range(2):
                        nc.tensor.matmul(
                            b_ps, uT[cc], tfull[:, 2 + cc:3 + cc],
                            start=(cc == 0), stop=(cc == 1),
                        )
                    if name == "q":
                        for r in range(4):
                            nc.vector.tensor_copy(bq4[D * r:D * (r + 1), :], b_ps)
                    elif name == "k":
                        for r in range(4):
                            nc.vector.tensor_copy(bk4[D * r:D * (r + 1), :], b_ps)
                    else:
                        nc.vector.tensor_copy(bv, b_ps)

                # w_out^T halves
                for hh in range(2):
                    wo_nat = pro_sb.tile([128, D], f32, tag="wo_nat")
                    nc.sync.dma_start(
                        out=wo_nat, in_=wo[128 * hh:128 * (hh + 1), :]
                    )
                    ps_t = pro_ps.tile([D, 128], f32, tag="tp", bufs=3)
                    nc.tensor.transpose(ps_t, wo_nat, ident)
                    nc.vector.tensor_copy(woT[hh], ps_t)  # fp32 psum -> bf16
                bo_dma = pro_sb.tile([128, 2], f32, tag="bo_dma")
                nc.sync.dma_start(out=bo_dma, in_=bo.rearrange("(f p) -> p f", p=128))
                nc.vector.tensor_copy(bo_t, bo_dma)

                # x loads (fp32 staging -> bf16 convert)
                for cc in range(2):
                    for qq in range(0, n_pos, 1024):
                        w = min(1024, n_pos - qq)
                        x_stage = pro_sb.tile([128, 1024], f32, tag="x_stage",
                                              bufs=3, name=f"x_stage_{cc}_{qq}")
                        nc.sync.dma_start(
                            out=x_stage[:, :w],
                            in_=xb[128 * cc:128 * (cc + 1), qq:qq + w],
                        )
                        nc.vector.tensor_copy(x_sb[cc][:, qq:qq + w], x_stage[:, :w])


            # ---------------- QKV projection + V^T build ----------------
            with tc.tile_pool(name="qkv_ps", bufs=2, space="PSUM") as qkv_ps, \
                 tc.tile_pool(name="vt_ps", bufs=2, space="PSUM") as vt_ps:
                for nt in range(0, n_pos, NT):
                    sl = slice(nt, nt + NT)
                    ps_q = qkv_ps.tile([128, NT], f32, tag="q")
                    ps_k = qkv_ps.tile([128, NT], f32, tag="k")
                    ps_v = qkv_ps.tile([D, NT], f32, tag="v")
                    for cc in range(2):
                        st, sp = (cc == 0), (cc == 1)
                        nc.tensor.matmul(ps_q, q4T[cc], x_sb[cc][:, sl], start=st, stop=sp)
                        nc.tensor.matmul(ps_k, k4T[cc], x_sb[cc][:, sl], start=st, stop=sp)
                        nc.tensor.matmul(ps_v, vT_w[cc], x_sb[cc][:, sl], start=st, stop=sp)
                    nc.vector.tensor_scalar_add(q4[:, sl], ps_q, bq4)
                    nc.vector.tensor_scalar_add(k4[:, sl], ps_k, bk4)
                    nc.vector.tensor_scalar_add(v_sb[:, sl], ps_v, bv)
                    # V^T tiles for the 4 j-tiles covered by this n-tile
                    for j in range(nt // JT, (nt + NT) // JT):
                        ps_vt = vt_ps.tile([128, D], bf16, tag="vt")
                        nc.tensor.transpose(
                            ps_vt, v_sb[:, j * JT:(j + 1) * JT], ident_bf[0:D, 0:D]
                        )
                        nc.vector.tensor_copy(vt_tiles[j][:, 0:D], ps_vt)
                        nc.vector.memset(vt_tiles[j][:, D:D + 1], 1.0)

            # ---------------- attention + output projection ----------------
            with tc.tile_pool(name="sc_ps", bufs=2, space="PSUM") as sc_ps, \
                 tc.tile_pool(name="u_ps", bufs=2, space="PSUM") as u_ps, \
                 tc.tile_pool(name="pt_sb", bufs=4) as pt_sb, \
                 tc.tile_pool(name="o_sb", bufs=2) as o_sb:
                def emit_proj(pit, onorm_t):
                    psl = slice(pit * NT, (pit + 1) * NT)
                    for hh in range(2):
                        ps_o = u_ps.tile([128, NT], f32, tag=("ua", "ub")[hh],
                                         name=f"ps_o_{pit}_{hh}")
                        nc.tensor.matmul(ps_o, woT[hh], onorm_t,
                                         start=True, stop=True)
                        o_out = o_sb.tile([128, NT], f32, tag="o_out",
                                          name=f"o_out_{pit}_{hh}")
                        nc.vector.tensor_scalar_add(o_out, ps_o, bo_t[:, hh:hh + 1])
                        nc.sync.dma_start(
                            out=out[128 * hh:128 * (hh + 1), psl], in_=o_out
                        )

                pending = None
                for it in range(n_itiles):
                    isl = slice(it * NT, (it + 1) * NT)
                    U_a = u_ps.tile([D + 1, NT], f32, tag="ua")
                    U_b = u_ps.tile([97, NT], f32, tag="ub")

                    def emit_scores(jp):
                        jA, jB = 2 * jp, 2 * jp + 1
                        S = sc_ps.tile([128, 2 * NT], f32, tag="sc", name=f"S_{jp}")
                        nc.tensor.matmul(
                            S[:, 0:NT],
                            k4[0:D, jA * JT:(jA + 1) * JT],
                            q4[0:D, isl],
                            start=True, stop=True, tile_position=(0, 0),
                        )
                        nc.tensor.matmul(
                            S[:, NT:2 * NT],
                            k4[D:2 * D, jB * JT:(jB + 1) * JT],
                            q4[D:2 * D, isl],
                            start=True, stop=True, tile_position=(32, 0),
                        )
                        PT = pt_sb.tile([128, 2 * NT], bf16, tag="pt",
                                        name=f"PT_{jp}")
                        nc.scalar.activation(PT, S, AF.Exp, scale=SCALE)
                        return PT

                    def emit_u(jp, PT):
                        jA, jB = 2 * jp, 2 * jp + 1
                        st, sp = (jp == 0), (jp == n_jpairs - 1)
                        nc.tensor.matmul(
                            U_a, vt_tiles[jA], PT[:, 0:NT],
                            start=st, stop=sp, tile_position=(0, 0),
                        )
                        nc.tensor.matmul(
                            U_b[64:64 + D + 1, :], vt_tiles[jB], PT[:, NT:2 * NT],
                            start=st, stop=sp, tile_position=(0, 64),
                        )

                    # software pipeline: scores run one stage ahead of U so
                    # the strict-FIFO PE queue never blocks a ready S matmul
                    # behind a U matmul whose exp isn't done yet.
                    prev_pt = emit_scores(0)
                    for jp in range(1, n_jpairs):
                        pt_cur = emit_scores(jp)
                        emit_u(jp - 1, prev_pt)
                        prev_pt = pt_cur
                    emit_u(n_jpairs - 1, prev_pt)
                    # combine halves + normalize (DVE/GpSimd, overlaps the
                    # next i-tile's j-loop); projection is deferred one i-tile
                    # so its PE matmuls never stall on this chain.
                    usum_b = o_sb.tile([D + 1, NT], f32, tag="usum_b")
                    nc.vector.tensor_copy(usum_b, U_b[64:64 + D + 1, :])
                    usum = o_sb.tile([D + 1, NT], f32, tag="usum")
                    nc.vector.tensor_add(usum, U_a, usum_b)
                    rcp = o_sb.tile([1, NT], f32, tag="rcp")
                    nc.vector.reciprocal(rcp, usum[D:D + 1, :])
                    rrep = o_sb.tile([D, NT], f32, tag="rrep")
                    nc.gpsimd.partition_broadcast(rrep, rcp)
                    onorm = o_sb.tile([D, NT], bf16, tag="onorm")
                    nc.vector.tensor_mul(onorm, usum[0:D, :], rrep)
                    if pending is not None:
                        emit_proj(*pending)
                    pending = (it, onorm)
                if pending is not None:
                    emit_proj(*pending)
    nc.compile()
    return nc


@functools.lru_cache(maxsize=2)
def _get_nc(n_pos=N_FULL):
    return _build_program(n_pos)


def _make_in_maps(x, time_emb, w_mlp, b_mlp, w_qkv, w_out, b_out, n_pos=N_FULL):
    x = np.ascontiguousarray(np.asarray(x, dtype=np.float32))
    time_emb = np.ascontiguousarray(np.asarray(time_emb, dtype=np.float32))
    w_mlp = np.ascontiguousarray(np.asarray(w_mlp, dtype=np.float32))
    b_mlp = np.ascontiguousarray(np.asarray(b_mlp, dtype=np.float32))
    w_qkv = np.ascontiguousarray(np.asarray(w_qkv, dtype=np.float32))
    w_out = np.ascontiguousarray(np.asarray(w_out, dtype=np.float32))
    b_out = np.ascontiguousarray(np.asarray(b_out, dtype=np.float32))

    b = x.shape[0]
    hid = HEADS * D
    in_maps = []
    for core in range(N_CORES):
        bb, hh = core // HEADS, core % HEADS
        in_maps.append({
            "xb": np.ascontiguousarray(
                x[bb].reshape(C, -1)[:, :n_pos]),
            "te": time_emb[bb],
            "w_mlp": w_mlp,
            "b_mlp": b_mlp,
            "wq": np.ascontiguousarray(w_qkv[D * hh:D * (hh + 1), :]),
            "wk": np.ascontiguousarray(w_qkv[hid + D * hh:hid + D * (hh + 1), :]),
            "wv": np.ascontiguousarray(
                w_qkv[2 * hid + D * hh:2 * hid + D * (hh + 1), :]),
            "wo": np.ascontiguousarray(w_out[:, D * hh:D * (hh + 1)]),
            "bo": b_out if hh == 0 else np.zeros_like(b_out),
        })
    return in_maps


def _install_ntff_hook():
    """Register the axon NTFF profile hook (the agent image's antenv lacks
    axon_hooks; replicate trn_boot's ctypes shim so trace=True works)."""
    import types
    import contextlib
    import ctypes

    try:
        from antenv.axon_hooks import get_axon_ntff_profile_hook  # noqa: F401
        return
    except ImportError:
        pass
    so_path = "/opt/axon/libaxon_pjrt.so"
    try:
        lib = ctypes.CDLL(so_path)
    except OSError:
        return
    if not hasattr(lib, "axon_start_nrt_profile"):
        return
    lib.axon_start_nrt_profile.argtypes = [
        ctypes.POINTER(ctypes.c_int64), ctypes.c_size_t]
    lib.axon_start_nrt_profile.restype = ctypes.c_int64
    lib.axon_stop_nrt_profile.argtypes = [ctypes.c_char_p]
    lib.axon_stop_nrt_profile.restype = ctypes.c_int64

    @contextlib.contextmanager
    def _hook(output_dir, device_ids):
        import jax
        jax.devices()
        if device_ids:
            ids = (ctypes.c_int64 * len(device_ids))(*device_ids)
            rc = lib.axon_start_nrt_profile(ids, len(device_ids))
        else:
            rc = lib.axon_start_nrt_profile(None, 0)
        if rc != 0:
            raise RuntimeError(f"axon_start_nrt_profile rc={rc}")
        try:
            yield
        finally:
            n = lib.axon_stop_nrt_profile(str(output_dir).encode())
            print(f"profile: {n} file(s) written to {output_dir}",
                  file=sys.stderr)

    import antenv
    mod = types.ModuleType("antenv.axon_hooks")
    mod.get_axon_ntff_profile_hook = lambda: _hook
    mod.set_axon_ntff_profile_hook = lambda h: None
    sys.modules["antenv.axon_hooks"] = mod
    antenv.axon_hooks = mod


def _run(inputs, trace=False, n_pos=N_FULL):
    from concourse.bass_utils import run_bass_kernel_spmd

    if trace:
        _install_ntff_hook()
    nc = _get_nc(n_pos)
    in_maps = _make_in_maps(**inputs, n_pos=n_pos)
    res = run_bass_kernel_spmd(
        nc, in_maps, core_ids=list(range(N_CORES)), trace=trace
    )
    return res


def _assemble(results, x_shape):
    b, c, h, w = x_shape
    out = np.zeros((b, c, h * w), dtype=np.float32)
    for core in range(N_CORES):
        bb = core // HEADS
        out[bb] += results[core]["out"]
    return out.reshape(b, c, h, w)


def kernel(x, time_emb, w_mlp, b_mlp, w_qkv, w_out, b_out):
    res = _run(dict(
        x=x, time_emb=time_emb, w_mlp=w_mlp, b_mlp=b_mlp,
        w_qkv=w_qkv, w_out=w_out, b_out=b_out,
    ))
    return _assemble(res.results, np.asarray(x).shape)


# revision 19
# speedup vs baseline: 1.0667x; 1.0038x over previous
"""Trainium2 Bass kernel: FiLM-conditioned 1x1-conv self-attention block.

Sharding: 8 cores = 2 batches x 4 heads. Each core computes one (batch, head)
pair end-to-end, producing a partial output projection [C, N]; the host sums
the 4 head partials per batch (b_out is added on the h==0 cores only).

Math notes (all exact re-associations of the reference):
  - FiLM: x~ = x*(1+scale) + shift  =>  W @ x~ = (W*diag(1+scale)) @ x + (W@shift)
    so the modulation is folded into the QKV weight columns + a rank-1 bias.
  - Attention is computed in transposed layout S^T[j,i] = sum_d k[d,j] q[d,i];
    softmax scale 1/sqrt(d) is folded into the exp activation's free scale.
    No max-subtraction (scores are O(5), exp is safe in fp32).
  - U = [V; 1]^T @ P^T accumulated over j gives both the unnormalized output
    (rows 0..32) and the softmax denominator (row 32) in one matmul chain.
"""

import functools
import sys

import numpy as np

if "/opt/trn_rl_repo" not in sys.path:
    sys.path.insert(0, "/opt/trn_rl_repo")

HEADS = 4
D = 32              # dim head
C = 256             # channels
TD = 512            # time embedding dim
SCALE = D ** -0.5
N_FULL = 4096       # 64*64 spatial positions
NT = 512            # query (i) tile
JT = 128            # key (j) tile
N_CORES = 8


def _build_program(n_pos=N_FULL):
    import concourse.bass as bass
    import concourse.tile as tile
    from concourse import bacc, mybir
    from concourse.masks import make_identity

    f32 = mybir.dt.float32
    bf16 = mybir.dt.bfloat16
    AF = mybir.ActivationFunctionType

    nc = bacc.Bacc("TRN2", debug=False)

    xb = nc.dram_tensor("xb", [C, n_pos], f32, kind="ExternalInput").ap()
    te = nc.dram_tensor("te", [TD], f32, kind="ExternalInput").ap()
    w_mlp = nc.dram_tensor("w_mlp", [TD, TD], f32, kind="ExternalInput").ap()
    b_mlp = nc.dram_tensor("b_mlp", [TD], f32, kind="ExternalInput").ap()
    wq = nc.dram_tensor("wq", [D, C], f32, kind="ExternalInput").ap()
    wk = nc.dram_tensor("wk", [D, C], f32, kind="ExternalInput").ap()
    wv = nc.dram_tensor("wv", [D, C], f32, kind="ExternalInput").ap()
    wo = nc.dram_tensor("wo", [C, D], f32, kind="ExternalInput").ap()
    bo = nc.dram_tensor("bo", [C], f32, kind="ExternalInput").ap()
    out = nc.dram_tensor("out", [C, n_pos], f32, kind="ExternalOutput").ap()

    n_itiles = n_pos // NT
    n_jtiles = n_pos // JT
    n_jpairs = n_jtiles // 2

    with tile.TileContext(nc) as tc:
        with tc.tile_pool(name="const", bufs=1) as const:
            ident = const.tile([128, 128], f32, name="ident")
            make_identity(nc, ident)
            ident_bf = const.tile([128, 128], bf16, name="ident_bf")
            make_identity(nc, ident_bf)

            # persistent big tiles
            x_sb = []
            for cc in range(2):
                xt = const.tile([128, n_pos], bf16, name=f"x_sb{cc}")
                x_sb.append(xt)
            q4 = const.tile([128, n_pos], bf16, name="q4")
            k4 = const.tile([128, n_pos], bf16, name="k4")
            v_sb = const.tile([D, n_pos], bf16, name="v_sb")

            # FiLM results
            tfull = const.tile([128, 4], f32, name="tfull")
            sc1 = const.tile([128, 2], f32, name="sc1")
            # folded qkv weights (transposed, q/k replicated 4x along M)
            q4T = [const.tile([128, 128], bf16, name=f"q4T{cc}") for cc in range(2)]
            k4T = [const.tile([128, 128], bf16, name=f"k4T{cc}") for cc in range(2)]
            vT_w = [const.tile([128, D], bf16, name=f"vT_w{cc}") for cc in range(2)]
            bq4 = const.tile([128, 1], f32, name="bq4")
            bk4 = const.tile([128, 1], f32, name="bk4")
            bv = const.tile([D, 1], f32, name="bv")
            woT = [const.tile([D, 128], bf16, name=f"woT{hh}") for hh in range(2)]
            bo_t = const.tile([128, 2], f32, name="bo_t")
            vt_tiles = [
                const.tile([128, D + 1], bf16, name=f"vt_{j}") for j in range(n_jtiles)
            ]

            # ---------------- prologue: FiLM + weight prep ----------------
            with tc.tile_pool(name="pro_sb", bufs=3) as pro_sb, \
                 tc.tile_pool(name="pro_ps", bufs=2, space="PSUM") as pro_ps:

                # silu(time_emb) chunked [128, 4] (c = 128*f + p)
                te_t = pro_sb.tile([128, 4], f32, tag="te_t")
                nc.sync.dma_start(out=te_t, in_=te.rearrange("(f p) -> p f", p=128))
                s_t = const.tile([128, 4], f32, name="s_t")
                sig_t = pro_sb.tile([128, 4], f32, tag="sig_t")
                nc.scalar.activation(sig_t, te_t, AF.Sigmoid)
                nc.vector.tensor_mul(s_t, te_t, sig_t)

                # W_mlp^T tiles via PE transpose
                wmT = [[None] * 4 for _ in range(4)]
                for cc4 in range(4):
                    for ot in range(4):
                        wm_nat = pro_sb.tile([128, 128], f32, tag="wm_nat")
                        nc.sync.dma_start(
                            out=wm_nat,
                            in_=w_mlp[ot * 128:(ot + 1) * 128,
                                      cc4 * 128:(cc4 + 1) * 128],
                        )
                        ps_t = pro_ps.tile([128, 128], f32, tag="tp", bufs=3)
                        nc.tensor.transpose(ps_t, wm_nat, ident)
                        wmT_t = pro_sb.tile([128, 128], f32, tag=f"wmT_{cc4}_{ot}",
                                            name=f"wmT_{cc4}_{ot}", bufs=1)
                        nc.vector.tensor_copy(wmT_t, ps_t)
                        wmT[cc4][ot] = wmT_t

                # t = W_mlp @ silu(te)  (o-tile at a time), + b_mlp
                bm_t = pro_sb.tile([128, 4], f32, tag="bm_t")
                nc.sync.dma_start(out=bm_t, in_=b_mlp.rearrange("(f p) -> p f", p=128))
                for ot in range(4):
                    t_ps = pro_ps.tile([128, 1], f32, tag="t_ps", bufs=2)
                    for cc4 in range(4):
                        nc.tensor.matmul(
                            t_ps, wmT[cc4][ot], s_t[:, cc4:cc4 + 1],
                            start=(cc4 == 0), stop=(cc4 == 3),
                        )
                    nc.vector.tensor_add(
                        tfull[:, ot:ot + 1], t_ps, bm_t[:, ot:ot + 1]
                    )
                # scale+1 for c-chunks (cols 0,1); shift is cols 2,3
                nc.vector.tensor_scalar_add(sc1, tfull[:, 0:2], 1.0)

                # W_qkv head slices -> transpose -> scale cols by (1+scale)
                for name, wsrc, dstT, nrep in (
                    ("q", wq, q4T, 4), ("k", wk, k4T, 4), ("v", wv, vT_w, 1),
                ):
                    w_nat = pro_sb.tile([D, C], f32, tag=f"w_nat_{name}", bufs=1)
                    nc.sync.dma_start(out=w_nat, in_=wsrc)
                    uT = [None, None]
                    for cc in range(2):
                        ps_t = pro_ps.tile([128, D], f32, tag="tp", bufs=3)
                        nc.tensor.transpose(
                            ps_t, w_nat[:, 128 * cc:128 * (cc + 1)],
                            ident[0:D, 0:D],
                        )
                        # unscaled copy (for the shift-bias matmul)
                        uT_t = pro_sb.tile([128, D], f32, tag=f"uT_{name}{cc}",
                                           name=f"uT_{name}{cc}", bufs=1)
                        nc.vector.tensor_copy(uT_t, ps_t)
                        uT[cc] = uT_t
                        # scaled (FiLM) copies, replicated nrep x along M
                        for r in range(nrep):
                            nc.vector.tensor_scalar_mul(
                                dstT[cc][:, D * r:D * (r + 1)], ps_t,
                                sc1[:, cc:cc + 1],
                            )
                    # bias_g = W_g @ shift (unscaled weights)
                    b_ps = pro_ps.tile([D, 1], f32, tag="b_ps", bufs=2)
                    for cc in range(2):
                        nc.tensor.matmul(
                            b_ps, uT[cc], tfull[:, 2 + cc:3 + cc],
                            start=(cc == 0), stop=(cc == 1),
                        )
                    if name == "q":
                        for r in range(4):
                            nc.vector.tensor_copy(bq4[D * r:D * (r + 1), :], b_ps)
                    elif name == "k":
                        for r in range(4):
                            nc.vector.tensor_copy(bk4[D * r:D * (r + 1), :], b_ps)
                    else:
                        nc.vector.tensor_copy(bv, b_ps)

                # w_out^T halves
                for hh in range(2):
                    wo_nat = pro_sb.tile([128, D], f32, tag="wo_nat")
                    nc.sync.dma_start(
                        out=wo_nat, in_=wo[128 * hh:128 * (hh + 1), :]
                    )
                    ps_t = pro_ps.tile([D, 128], f32, tag="tp", bufs=3)
                    nc.tensor.transpose(ps_t, wo_nat, ident)
                    nc.vector.tensor_copy(woT[hh], ps_t)  # fp32 psum -> bf16
                bo_dma = pro_sb.tile([128, 2], f32, tag="bo_dma")
                nc.sync.dma_start(out=bo_dma, in_=bo.rearrange("(f p) -> p f", p=128))
                nc.vector.tensor_copy(bo_t, bo_dma)

                # x loads (fp32 staging -> bf16 convert)
                for cc in range(2):
                    for qq in range(0, n_pos, 1024):
                        w = min(1024, n_pos - qq)
                        x_stage = pro_sb.tile([128, 1024], f32, tag="x_stage",
                                              bufs=3, name=f"x_stage_{cc}_{qq}")
                        nc.sync.dma_start(
                            out=x_stage[:, :w],
                            in_=xb[128 * cc:128 * (cc + 1), qq:qq + w],
                        )
                        nc.vector.tensor_copy(x_sb[cc][:, qq:qq + w], x_stage[:, :w])


            # ---------------- QKV projection + V^T build ----------------
            with tc.tile_pool(name="qkv_ps", bufs=2, space="PSUM") as qkv_ps, \
                 tc.tile_pool(name="vt_ps", bufs=2, space="PSUM") as vt_ps:
                for nt in range(0, n_pos, NT):
                    sl = slice(nt, nt + NT)
                    ps_q = qkv_ps.tile([128, NT], f32, tag="q")
                    ps_k = qkv_ps.tile([128, NT], f32, tag="k")
                    ps_v = qkv_ps.tile([D, NT], f32, tag="v")
                    for cc in range(2):
                        st, sp = (cc == 0), (cc == 1)
                        nc.tensor.matmul(ps_q, q4T[cc], x_sb[cc][:, sl], start=st, stop=sp)
                        nc.tensor.matmul(ps_k, k4T[cc], x_sb[cc][:, sl], start=st, stop=sp)
                        nc.tensor.matmul(ps_v, vT_w[cc], x_sb[cc][:, sl], start=st, stop=sp)
                    nc.vector.tensor_scalar_add(q4[:, sl], ps_q, bq4)
                    nc.vector.tensor_scalar_add(k4[:, sl], ps_k, bk4)
                    nc.vector.tensor_scalar_add(v_sb[:, sl], ps_v, bv)
                    # V^T tiles for the 4 j-tiles covered by this n-tile
                    for j in range(nt // JT, (nt + NT) // JT):
                        ps_vt = vt_ps.tile([128, D], bf16, tag="vt")
                        nc.tensor.transpose(
                            ps_vt, v_sb[:, j * JT:(j + 1) * JT], ident_bf[0:D, 0:D]
                        )
                        nc.vector.tensor_copy(vt_tiles[j][:, 0:D], ps_vt)
                        nc.vector.memset(vt_tiles[j][:, D:D + 1], 1.0)

            # ---------------- attention + output projection ----------------
            with tc.tile_pool(name="sc_ps", bufs=2, space="PSUM") as sc_ps, \
                 tc.tile_pool(name="u_ps", bufs=2, space="PSUM") as u_ps, \
                 tc.tile_pool(name="aux_ps", bufs=2, space="PSUM") as aux_ps, \
                 tc.tile_pool(name="pt_sb", bufs=4) as pt_sb, \
                 tc.tile_pool(name="o_sb", bufs=2) as o_sb:
                def emit_proj(pit, onorm_t):
                    psl = slice(pit * NT, (pit + 1) * NT)
                    for hh in range(2):
                        ps_o = aux_ps.tile([128, NT], f32, tag="aux",
                                           name=f"ps_o_{pit}_{hh}")
                        nc.tensor.matmul(ps_o, woT[hh], onorm_t,
                                         start=True, stop=True)
                        o_out = o_sb.tile([128, NT], f32, tag="o_out",
                                          name=f"o_out_{pit}_{hh}")
                        nc.vector.tensor_scalar_add(o_out, ps_o, bo_t[:, hh:hh + 1])
                        nc.sync.dma_start(
                            out=out[128 * hh:128 * (hh + 1), psl], in_=o_out
                        )

                pending = None
                for it in range(n_itiles):
                    isl = slice(it * NT, (it + 1) * NT)
                    U = u_ps.tile([97, NT], f32, tag="u")

                    def emit_scores(jp):
                        jA, jB = 2 * jp, 2 * jp + 1
                        S = sc_ps.tile([128, 2 * NT], f32, tag="sc", name=f"S_{jp}")
                        nc.tensor.matmul(
                            S[:, 0:NT],
                            k4[0:D, jA * JT:(jA + 1) * JT],
                            q4[0:D, isl],
                            start=True, stop=True, tile_position=(0, 0),
                        )
                        nc.tensor.matmul(
                            S[:, NT:2 * NT],
                            k4[D:2 * D, jB * JT:(jB + 1) * JT],
                            q4[D:2 * D, isl],
                            start=True, stop=True, tile_position=(32, 0),
                        )
                        PT = pt_sb.tile([128, 2 * NT], bf16, tag="pt",
                                        name=f"PT_{jp}")
                        nc.scalar.activation(PT, S, AF.Exp, scale=SCALE)
                        return PT

                    def emit_u(jp, PT):
                        jA, jB = 2 * jp, 2 * jp + 1
                        st, sp = (jp == 0), (jp == n_jpairs - 1)
                        nc.tensor.matmul(
                            U[0:D + 1, :], vt_tiles[jA], PT[:, 0:NT],
                            start=st, stop=sp, tile_position=(0, 0),
                            skip_group_check=True,
                        )
                        nc.tensor.matmul(
                            U[64:64 + D + 1, :], vt_tiles[jB], PT[:, NT:2 * NT],
                            start=st, stop=sp, tile_position=(0, 64),
                            skip_group_check=True,
                        )

                    # software pipeline: scores run one stage ahead of U so
                    # the strict-FIFO PE queue never blocks a ready S matmul
                    # behind a U matmul whose exp isn't done yet.
                    prev_pt = emit_scores(0)
                    for jp in range(1, n_jpairs):
                        pt_cur = emit_scores(jp)
                        emit_u(jp - 1, prev_pt)
                        prev_pt = pt_cur
                    emit_u(n_jpairs - 1, prev_pt)
                    # combine halves + normalize (DVE/GpSimd, overlaps the
                    # next i-tile's j-loop); projection is deferred one i-tile
                    # so its PE matmuls never stall on this chain.
                    usum_b = o_sb.tile([D + 1, NT], f32, tag="usum_b")
                    nc.vector.tensor_copy(usum_b, U[64:64 + D + 1, :])
                    usum = o_sb.tile([D + 1, NT], f32, tag="usum")
                    nc.vector.tensor_add(usum, U[0:D + 1, :], usum_b)
                    rcp = o_sb.tile([1, NT], f32, tag="rcp")
                    nc.vector.reciprocal(rcp, usum[D:D + 1, :])
                    rrep = o_sb.tile([D, NT], f32, tag="rrep")
                    nc.gpsimd.partition_broadcast(rrep, rcp)
                    onorm = o_sb.tile([D, NT], bf16, tag="onorm")
                    nc.vector.tensor_mul(onorm, usum[0:D, :], rrep)
                    if pending is not None:
                        emit_proj(*pending)
                    pending = (it, onorm)
                if pending is not None:
                    emit_proj(*pending)
    nc.compile()
    return nc


@functools.lru_cache(maxsize=2)
def _get_nc(n_pos=N_FULL):
    return _build_program(n_pos)


def _make_in_maps(x, time_emb, w_mlp, b_mlp, w_qkv, w_out, b_out, n_pos=N_FULL):
    x = np.ascontiguousarray(np.asarray(x, dtype=np.float32))
    time_emb = np.ascontiguousarray(np.asarray(time_emb, dtype=np.float32))
    w_mlp = np.ascontiguousarray(np.asarray(w_mlp, dtype=np.float32))
    b_mlp = np.ascontiguousarray(np.asarray(b_mlp, dtype=np.float32))
    w_qkv = np.ascontiguousarray(np.asarray(w_qkv, dtype=np.float32))
    w_out = np.ascontiguousarray(np.asarray(w_out, dtype=np.float32))
    b_out = np.ascontiguousarray(np.asarray(b_out, dtype=np.float32))

    b = x.shape[0]
    hid = HEADS * D
    in_maps = []
    for core in range(N_CORES):
        bb, hh = core // HEADS, core % HEADS
        in_maps.append({
            "xb": np.ascontiguousarray(
                x[bb].reshape(C, -1)[:, :n_pos]),
            "te": time_emb[bb],
            "w_mlp": w_mlp,
            "b_mlp": b_mlp,
            "wq": np.ascontiguousarray(w_qkv[D * hh:D * (hh + 1), :]),
            "wk": np.ascontiguousarray(w_qkv[hid + D * hh:hid + D * (hh + 1), :]),
            "wv": np.ascontiguousarray(
                w_qkv[2 * hid + D * hh:2 * hid + D * (hh + 1), :]),
            "wo": np.ascontiguousarray(w_out[:, D * hh:D * (hh + 1)]),
            "bo": b_out if hh == 0 else np.zeros_like(b_out),
        })
    return in_maps


def _install_ntff_hook():
    """Register the axon NTFF profile hook (the agent image's antenv lacks
    axon_hooks; replicate trn_boot's ctypes shim so trace=True works)."""
    import types
    import contextlib
    import ctypes

    try:
        from antenv.axon_hooks import get_axon_ntff_profile_hook  # noqa: F401
        return
    except ImportError:
        pass
    so_path = "/opt/axon/libaxon_pjrt.so"
    try:
        lib = ctypes.CDLL(so_path)
    except OSError:
        return
    if not hasattr(lib, "axon_start_nrt_profile"):
        return
    lib.axon_start_nrt_profile.argtypes = [
        ctypes.POINTER(ctypes.c_int64), ctypes.c_size_t]
    lib.axon_start_nrt_profile.restype = ctypes.c_int64
    lib.axon_stop_nrt_profile.argtypes = [ctypes.c_char_p]
    lib.axon_stop_nrt_profile.restype = ctypes.c_int64

    @contextlib.contextmanager
    def _hook(output_dir, device_ids):
        import jax
        jax.devices()
        if device_ids:
            ids = (ctypes.c_int64 * len(device_ids))(*device_ids)
            rc = lib.axon_start_nrt_profile(ids, len(device_ids))
        else:
            rc = lib.axon_start_nrt_profile(None, 0)
        if rc != 0:
            raise RuntimeError(f"axon_start_nrt_profile rc={rc}")
        try:
            yield
        finally:
            n = lib.axon_stop_nrt_profile(str(output_dir).encode())
            print(f"profile: {n} file(s) written to {output_dir}",
                  file=sys.stderr)

    import antenv
    mod = types.ModuleType("antenv.axon_hooks")
    mod.get_axon_ntff_profile_hook = lambda: _hook
    mod.set_axon_ntff_profile_hook = lambda h: None
    sys.modules["antenv.axon_hooks"] = mod
    antenv.axon_hooks = mod


def _run(inputs, trace=False, n_pos=N_FULL):
    from concourse.bass_utils import run_bass_kernel_spmd

    if trace:
        _install_ntff_hook()
    nc = _get_nc(n_pos)
    in_maps = _make_in_maps(**inputs, n_pos=n_pos)
    res = run_bass_kernel_spmd(
        nc, in_maps, core_ids=list(range(N_CORES)), trace=trace
    )
    return res


def _assemble(results, x_shape):
    b, c, h, w = x_shape
    out = np.zeros((b, c, h * w), dtype=np.float32)
    for core in range(N_CORES):
        bb = core // HEADS
        out[bb] += results[core]["out"]
    return out.reshape(b, c, h, w)


def kernel(x, time_emb, w_mlp, b_mlp, w_qkv, w_out, b_out):
    res = _run(dict(
        x=x, time_emb=time_emb, w_mlp=w_mlp, b_mlp=b_mlp,
        w_qkv=w_qkv, w_out=w_out, b_out=b_out,
    ))
    return _assemble(res.results, np.asarray(x).shape)
